# revision 2
# baseline (speedup 1.0000x reference)
"""Trainium2 Bass kernel for nn_MinusSpan.

Per (batch, span) with span (i, j):
  out = [fwd[j] - fwd[i-1], bwd[i] - bwd[j+1], fwd[i-1], bwd[j+1]]
where fwd/bwd are the two halves of the feature dim, fwd[i-1] is zero when
i == 0, bwd[j+1] is zero when j+1 >= T, and the whole row is zero for
padding spans (i == 0 and j == 0).

Strategy: data-parallel over the batch dim, 2 batch rows per core on 8
cores. The input shard is viewed as half-rows [BPC*T*2, 512] with one extra
zero half-row appended. All index arithmetic (i-1, j+1, clipping, masks)
is folded into host-computed int16 half-row indices; masked gathers point
at the zero half-row. The device kernel is then: Q7 dma_gather of 4
half-rows per span directly into the packed output layout, one in-place
vector subtract, contiguous store.
"""
import numpy as np
from contextlib import ExitStack

import concourse.bass as bass
import concourse.tile as tile
from concourse import bacc, mybir
from concourse.bass_utils import run_bass_kernel_spmd

B, T, D = 16, 2048, 1024
H = D // 2              # 512 floats per half-row (2 KiB)
N = 256                 # spans per batch row
NCORES = 8
BPC = B // NCORES       # batch rows per core
R = BPC * T * 2         # half-rows per core shard
ZR = R                  # index of the appended all-zero half-row
NBLK = BPC * 2          # chunks per core: 128 spans x 4 kinds each
NIDX = 128 * 4          # gathered half-rows per chunk
IW = NIDX // 16         # idx columns per chunk in the wrapped layout

_NC = None


def _build():
    """Build + compile the per-core Bass program (identical on all cores)."""
    nc = bacc.Bacc("TRN2", target_bir_lowering=False, debug=False,
                   num_devices=NCORES)
    x = nc.dram_tensor("x", [R + 1, H], mybir.dt.float32, kind="ExternalInput")
    idx = nc.dram_tensor("idx", [128, NBLK * IW], mybir.dt.int16,
                         kind="ExternalInput")
    out = nc.dram_tensor("out", [BPC * N, 4 * H], mybir.dt.float32,
                         kind="ExternalOutput")

    with tile.TileContext(nc) as tc, ExitStack() as ctx:
        ipool = ctx.enter_context(tc.tile_pool(name="idxp", bufs=1))
        pool = ctx.enter_context(tc.tile_pool(name="work", bufs=4))

        idx_t = ipool.tile([128, NBLK * IW], mybir.dt.int16)
        nc.sync.dma_start(idx_t[:], idx[:])

        for blk in range(NBLK):  # blk = b*2 + cb; spans cb*128..cb*128+127
            o = pool.tile([128, 4, H], mybir.dt.float32)
            # gathered row n -> o[n%128, n//128, :]; n = kind*128 + p
            nc.gpsimd.dma_gather(
                o[:],
                x[:],
                idx_t[:, blk * IW:(blk + 1) * IW],
                NIDX,
                NIDX,
                H,
            )
            of = o[:].rearrange("p k e -> p (k e)")
            # [t1-t2, t3-t4] into the first half; [t2, t4] stays in the second
            nc.vector.tensor_tensor(
                out=of[:, 0:2 * H], in0=of[:, 0:2 * H], in1=of[:, 2 * H:4 * H],
                op=mybir.AluOpType.subtract,
            )
            nc.sync.dma_start(out[blk * 128:(blk + 1) * 128, :], of[:])

    nc.compile()
    return nc


def _prep_core(input_c: np.ndarray, span_c: np.ndarray) -> dict:
    """Host-side shard prep: half-row view of the input + gather indices."""
    xs = np.ascontiguousarray(input_c, dtype=np.float32).reshape(R, H)
    x = np.concatenate([xs, np.zeros((1, H), np.float32)], axis=0)

    i = span_c[..., 0].astype(np.int64)   # [BPC, N]
    j = span_c[..., 1].astype(np.int64)
    base = (np.arange(BPC, dtype=np.int64) * T)[:, None]
    t1 = 2 * (base + j)                                    # fwd[j]
    t3 = 2 * (base + i) + 1                                # bwd[i]
    t2 = np.where(i >= 1, 2 * (base + i - 1), ZR)          # fwd[i-1] | 0
    t4 = np.where(j + 1 < T, 2 * (base + j + 1) + 1, ZR)   # bwd[j+1] | 0
    kinds = np.stack([t1, t3, t2, t4], axis=-1)            # [BPC, N, 4]
    kinds[(i == 0) & (j == 0)] = ZR                        # padding spans
    # semantic order per chunk blk=(b,cb): n = kind*128 + p for span cb*128+p
    sem = (kinds.reshape(BPC, 2, 128, 4)       # [b, cb, p, kind]
           .transpose(0, 1, 3, 2)              # [b, cb, kind, p]
           .reshape(NBLK, NIDX))               # [blk, n]
    # wrapped layout: w[p16, s] = sem[s*16 + p16], replicated to 128 parts
    w = sem.reshape(NBLK, IW, 16).transpose(0, 2, 1)       # [blk, 16, IW]
    w = np.tile(w, (1, 8, 1))                              # [blk, 128, IW]
    idx = (w.transpose(1, 0, 2).reshape(128, NBLK * IW)).astype(np.int16)
    return {"x": x, "idx": idx}


def _run(inputs: dict, trace: bool = False, **kw):
    global _NC
    if _NC is None:
        _NC = _build()
    inp = np.asarray(inputs["input"])
    spans = np.asarray(inputs["span_idxs"])
    in_maps = [
        _prep_core(inp[c * BPC:(c + 1) * BPC], spans[c * BPC:(c + 1) * BPC])
        for c in range(NCORES)
    ]
    res = run_bass_kernel_spmd(_NC, in_maps, core_ids=list(range(NCORES)),
                               trace=trace, **kw)
    full = np.concatenate(
        [res.results[c]["out"].reshape(BPC, N, 4 * H) for c in range(NCORES)],
        axis=0,
    )
    return full, res


def kernel(input: np.ndarray, span_idxs: np.ndarray) -> np.ndarray:
    full, _ = _run({"input": input, "span_idxs": span_idxs})
    return full
